# revision 22
# baseline (speedup 1.0000x reference)
"""Trainium2 Bass kernel for linear attention over external memory.

Computes out = x @ (keys^T @ vals) for
  x [4, 2048, 1024] f32, keys/vals [65536, 1024] f32.

Sharding across 8 NeuronCores: keys/vals sharded along the memory dim M
(8192 rows per core); each core computes a partial kv = keys_s^T @ vals_s,
AllReduces kv in bf16 (split in two column halves so the first AllReduce
overlaps remaining work), then computes its token shard of x @ kv
(x sharded by token, 1024 rows per core).

Stage 2 runs in float32r (TF32-like, full PE rate for moving dim >= 256)
directly on the DMA'd f32 data — no cast step. kv is accumulated in
PSUM per group of 8 k-chunks and drained into an SBUF f32 accumulator.
"""

import numpy as np

# Problem shapes (hardcoded per contract).
B, S, D = 4, 2048, 1024
M = 65536
NCORES = 8
P = 128
T = (B * S) // NCORES          # 1024 tokens per core
KM = M // NCORES               # 8192 memory rows per core
NC_ = KM // P                  # 64 k-chunks
G = 8                          # chunks per PSUM accumulation group
NG = NC_ // G                  # 8 groups
DB = D // P                    # 8 d-blocks
HALF = D // 2                  # 512
TCH = T // P                   # 8 token chunks

_CACHE = {}


def _build_nc():
    import concourse.bacc as bacc
    import concourse.tile as tile
    from concourse import mybir
    from concourse.masks import make_identity

    f32 = mybir.dt.float32
    f32r = mybir.dt.float32r
    bf16 = mybir.dt.bfloat16
    ACT_COPY = mybir.ActivationFunctionType.Copy

    nc = bacc.Bacc("TRN2", target_bir_lowering=False, debug=False,
                   num_devices=NCORES)

    xs_d = nc.dram_tensor("xs", [T, D], f32, kind="ExternalInput")
    ks_d = nc.dram_tensor("ks", [KM, D], f32r, kind="ExternalInput")
    vs_d = nc.dram_tensor("vs", [KM, D], f32r, kind="ExternalInput")
    out_d = nc.dram_tensor("out", [T, D], f32, kind="ExternalOutput")

    ks_r = ks_d.ap().rearrange("(c p) n -> c p n", p=P)   # [64, 128, 1024]
    vs_r = vs_d.ap().rearrange("(c p) n -> c p n", p=P)
    xs_r = xs_d.ap().rearrange("(c p) n -> c p n", p=P)   # [8, 128, 1024]

    with tile.TileContext(nc) as tc:
        with (
            tc.tile_pool(name="const", bufs=1) as const,
            tc.tile_pool(name="kfp", bufs=11) as kfp,
            tc.tile_pool(name="vfp", bufs=11) as vfp,
            tc.tile_pool(name="accp", bufs=2 * DB) as accp,
            tc.tile_pool(name="xstage", bufs=TCH) as xstage,
            tc.tile_pool(name="xtp", bufs=DB) as xtp,
            tc.tile_pool(name="kvio", bufs=2) as kvio,
            tc.tile_pool(name="outp", bufs=3) as outp,
            tc.tile_pool(name="ps", bufs=8, space="PSUM") as ps,
            tc.tile_pool(name="dram", bufs=6, space="DRAM") as dram,
        ):
            ident = const.tile([P, P], f32)
            make_identity(nc, ident)

            # Warm-up collective: arms the ncfw collective stream so the
            # first real AllReduce trigger doesn't pay the ~11us wake-up.
            warm = const.tile([P, 16], bf16)
            nc.gpsimd.memset(warm[:], 0.0)
            warm_in = dram.tile([P, 16], bf16, name="warm_in")
            warm_out = dram.tile([P, 16], bf16, name="warm_out",
                                 addr_space="Shared")
            nc.gpsimd.dma_start(out=warm_in[:], in_=warm[:])
            nc.gpsimd.collective_compute(
                "AllReduce",
                mybir.AluOpType.add,
                replica_groups=[list(range(NCORES))],
                ins=[warm_in.opt()],
                outs=[warm_out.opt()],
            )

            # kv accumulator: tile (h*DB+j) holds kv[j*128:(j+1)*128,
            # h*512:(h+1)*512] as [128, 512] f32.
            acc = [accp.tile([P, HALF], f32, name=f"acc{i}", tag="acc")
                   for i in range(2 * DB)]
            for i in range(2 * DB):
                nc.vector.memset(acc[i][:], 0.0)

            # ---- stage 2: kv partial, grouped PSUM accumulation ----
            # keys loads on sync, vals loads on gpsimd; first chunks
            # row-split across two DMAs to shorten the start ramp.
            xf_tiles = []
            for g in range(NG):
                kf = []   # per chunk: (tile_cols_0_512, tile_cols_512_1024)
                vf = []
                for c in range(G):
                    if g == 0 and c < 2:
                        # Separate half-tiles for the first chunks:
                        # independent deps, so the first matmul only
                        # waits on a 256KB transfer.
                        kta = kfp.tile([P, HALF], f32r, name="kta",
                                       tag="kth", bufs=4)
                        ktb = kfp.tile([P, HALF], f32r, name="ktb",
                                       tag="kth", bufs=4)
                        vta = vfp.tile([P, HALF], f32r, name="vta",
                                       tag="vth", bufs=4)
                        vtb = vfp.tile([P, HALF], f32r, name="vtb",
                                       tag="vth", bufs=4)
                        nc.sync.dma_start(out=kta[:], in_=ks_r[c][:, :HALF])
                        nc.sync.dma_start(out=vta[:], in_=vs_r[c][:, :HALF])
                        nc.sync.dma_start(out=ktb[:], in_=ks_r[c][:, HALF:])
                        nc.sync.dma_start(out=vtb[:], in_=vs_r[c][:, HALF:])
                        kf.append((kta, ktb))
                        vf.append((vta, vtb))
                    else:
                        kt = kfp.tile([P, D], f32r, name="kt", tag="kt")
                        vt = vfp.tile([P, D], f32r, name="vt", tag="vt")
                        nc.sync.dma_start(out=kt[:], in_=ks_r[g * G + c])
                        nc.sync.dma_start(out=vt[:], in_=vs_r[g * G + c])
                        kf.append((kt, kt))
                        vf.append((vt, vt))
                # Sub-passes by d-half: psum holds (4 d-blocks x 2
                # e-halves) = 8 banks, and the two e-half matmuls per
                # (c, j) share one stationary load.
                for dh in range(2):
                    jlist = range(dh * 4, dh * 4 + 4)
                    pst = {(j, h): ps.tile([P, HALF], f32,
                                           name=f"kv{dh}_{j}_{h}",
                                           tag="ps")
                           for j in jlist for h in range(2)}
                    # c-outer keeps the ramp short; j-outer in the last
                    # group spreads the final accumulations so the
                    # AllReduce triggers as early as possible.
                    order = (
                        [(j, c) for j in jlist for c in range(G)]
                        if g == NG - 1 else
                        [(j, c) for c in range(G) for j in jlist])
                    for j, c in order:
                        split = kf[c][0] is not kf[c][1]
                        kt_h = kf[c][j // 4]
                        koff = (j % 4) * P if split else j * P
                        for h in range(2):
                            if split:
                                vt_h, voff = vf[c][h], 0
                            else:
                                vt_h, voff = vf[c][0], h * HALF
                            nc.tensor.matmul(
                                pst[(j, h)][:],
                                kt_h[:, koff:koff + P],
                                vt_h[:, voff:voff + HALF],
                                start=(c == 0), stop=(c == G - 1))
                    for j in jlist:
                        for h in range(2):
                            nc.vector.tensor_tensor(
                                out=acc[h * DB + j][:],
                                in0=pst[(j, h)][:],
                                in1=acc[h * DB + j][:],
                                op=mybir.AluOpType.add)
            # x loads at the tail of the load stream.
            for i in range(TCH):
                xf = xstage.tile([P, D], f32, name="xf", tag="xf")
                nc.sync.dma_start(out=xf[:], in_=xs_r[i])
                xf_tiles.append(xf)

            # ---- AllReduce kv, split by column half ----
            # Cast + bounce each d-block slice as soon as its final
            # accumulation lands, so the collective triggers early.
            kvr = []
            for h in range(2):
                kvev = kvio.tile([P, DB * HALF], bf16, name=f"kvev{h}",
                                 tag="kvio")
                bounce_in = dram.tile([P, DB * HALF], bf16,
                                      name=f"bin{h}", tag="bin")
                bounce_out = dram.tile([P, DB * HALF], bf16,
                                       name=f"bout{h}", tag="bout",
                                       addr_space="Shared")
                for j in range(DB):
                    sl = slice(j * HALF, (j + 1) * HALF)
                    nc.scalar.activation(
                        kvev[:, sl], acc[h * DB + j][:], ACT_COPY)
                    nc.gpsimd.dma_start(out=bounce_in[:, sl],
                                        in_=kvev[:, sl])
                nc.gpsimd.collective_compute(
                    "AllReduce",
                    mybir.AluOpType.add,
                    replica_groups=[list(range(NCORES))],
                    ins=[bounce_in.opt()],
                    outs=[bounce_out.opt()],
                )
                kvr.append(bounce_out)

            # ---- x: PE-transpose, cast to bf16 (fills AR wait) ----
            xT = [xtp.tile([P, T], bf16, name=f"xT{j}", tag="xT")
                  for j in range(DB)]
            for i in range(TCH):
                xf = xf_tiles[i]
                for j in range(DB):
                    pst = ps.tile([P, P], f32, name="pst", tag="ps")
                    nc.tensor.transpose(
                        pst[:], xf[:, j * P:(j + 1) * P], ident[:])
                    nc.vector.tensor_copy(
                        out=xT[j][:, i * P:(i + 1) * P], in_=pst[:])

            # ---- stage 4: out = x @ kv, per column half ----
            for h in range(2):
                kvh = kvio.tile([P, DB * HALF], bf16, name=f"kvr{h}",
                                tag="kvio")
                for j in range(DB):
                    sl = slice(j * HALF, (j + 1) * HALF)
                    nc.gpsimd.dma_start(out=kvh[:, sl], in_=kvr[h][:, sl])
                for i in range(TCH):
                    po = ps.tile([P, HALF], f32, name="po", tag="ps")
                    for j in range(DB):
                        nc.tensor.matmul(
                            po[:],
                            xT[j][:, i * P:(i + 1) * P],
                            kvh[:, j * HALF:(j + 1) * HALF],
                            start=(j == 0), stop=(j == DB - 1))
                    ob = outp.tile([P, HALF], f32, name="ob", tag="ob")
                    nc.scalar.activation(ob[:], po[:], ACT_COPY)
                    nc.scalar.dma_start(
                        out=out_d.ap()[i * P:(i + 1) * P,
                                       h * HALF:(h + 1) * HALF],
                        in_=ob[:])

    nc.compile()
    return nc


def _get_nc():
    if "nc" not in _CACHE:
        _CACHE["nc"] = _build_nc()
    return _CACHE["nc"]


def kernel(**inputs):
    from concourse.bass_utils import run_bass_kernel_spmd

    x = np.ascontiguousarray(np.asarray(inputs["x"], dtype=np.float32))
    keys = np.ascontiguousarray(np.asarray(inputs["keys"], dtype=np.float32))
    vals = np.ascontiguousarray(np.asarray(inputs["vals"], dtype=np.float32))
    xf = x.reshape(B * S, D)

    nc = _get_nc()
    in_maps = []
    for c in range(NCORES):
        in_maps.append({
            "xs": xf[c * T:(c + 1) * T],
            "ks": keys[c * KM:(c + 1) * KM],
            "vs": vals[c * KM:(c + 1) * KM],
        })
    res = run_bass_kernel_spmd(nc, in_maps, list(range(NCORES)))
    out = np.concatenate([res.results[c]["out"] for c in range(NCORES)],
                         axis=0)
    return out.reshape(B, S, D).astype(np.float32)


# revision 23
# speedup vs baseline: 1.0301x; 1.0301x over previous
"""Trainium2 Bass kernel for linear attention over external memory.

Computes out = x @ (keys^T @ vals) for
  x [4, 2048, 1024] f32, keys/vals [65536, 1024] f32.

Sharding across 8 NeuronCores: keys/vals sharded along the memory dim M
(8192 rows per core); each core computes a partial kv = keys_s^T @ vals_s,
AllReduces kv in bf16 (split in two column halves so the first AllReduce
overlaps remaining work), then computes its token shard of x @ kv
(x sharded by token, 1024 rows per core).

Stage 2 runs in float32r (TF32-like, full PE rate for moving dim >= 256)
directly on the DMA'd f32 data — no cast step. kv is accumulated in
PSUM per group of 8 k-chunks and drained into an SBUF f32 accumulator.
"""

import numpy as np

# Problem shapes (hardcoded per contract).
B, S, D = 4, 2048, 1024
M = 65536
NCORES = 8
P = 128
T = (B * S) // NCORES          # 1024 tokens per core
KM = M // NCORES               # 8192 memory rows per core
NC_ = KM // P                  # 64 k-chunks
G = 8                          # chunks per PSUM accumulation group
NG = NC_ // G                  # 8 groups
DB = D // P                    # 8 d-blocks
HALF = D // 2                  # 512
TCH = T // P                   # 8 token chunks

_CACHE = {}


def _build_nc():
    import concourse.bacc as bacc
    import concourse.tile as tile
    from concourse import mybir
    from concourse.masks import make_identity

    f32 = mybir.dt.float32
    f32r = mybir.dt.float32r
    bf16 = mybir.dt.bfloat16
    ACT_COPY = mybir.ActivationFunctionType.Copy

    nc = bacc.Bacc("TRN2", target_bir_lowering=False, debug=False,
                   num_devices=NCORES)

    xs_d = nc.dram_tensor("xs", [T, D], f32, kind="ExternalInput")
    ks_d = nc.dram_tensor("ks", [KM, D], f32r, kind="ExternalInput")
    vs_d = nc.dram_tensor("vs", [KM, D], f32r, kind="ExternalInput")
    out_d = nc.dram_tensor("out", [T, D], f32, kind="ExternalOutput")

    ks_r = ks_d.ap().rearrange("(c p) n -> c p n", p=P)   # [64, 128, 1024]
    vs_r = vs_d.ap().rearrange("(c p) n -> c p n", p=P)
    xs_r = xs_d.ap().rearrange("(c p) n -> c p n", p=P)   # [8, 128, 1024]

    with tile.TileContext(nc) as tc:
        with (
            tc.tile_pool(name="const", bufs=1) as const,
            tc.tile_pool(name="kfp", bufs=11) as kfp,
            tc.tile_pool(name="vfp", bufs=11) as vfp,
            tc.tile_pool(name="accp", bufs=2 * DB) as accp,
            tc.tile_pool(name="xstage", bufs=TCH) as xstage,
            tc.tile_pool(name="xtp", bufs=DB) as xtp,
            tc.tile_pool(name="kvio", bufs=2) as kvio,
            tc.tile_pool(name="outp", bufs=3) as outp,
            tc.tile_pool(name="ps", bufs=8, space="PSUM") as ps,
            tc.tile_pool(name="dram", bufs=6, space="DRAM") as dram,
        ):
            ident = const.tile([P, P], f32)
            make_identity(nc, ident)

            # Warm-up collective: arms the ncfw collective stream so the
            # first real AllReduce trigger doesn't pay the ~11us wake-up.
            warm = const.tile([P, 16], bf16)
            nc.gpsimd.memset(warm[:], 0.0)
            warm_in = dram.tile([P, 16], bf16, name="warm_in")
            warm_out = dram.tile([P, 16], bf16, name="warm_out",
                                 addr_space="Shared")
            nc.gpsimd.dma_start(out=warm_in[:], in_=warm[:])
            nc.gpsimd.collective_compute(
                "AllReduce",
                mybir.AluOpType.add,
                replica_groups=[list(range(NCORES))],
                ins=[warm_in.opt()],
                outs=[warm_out.opt()],
            )

            # kv accumulator: tile (h*DB+j) holds kv[j*128:(j+1)*128,
            # h*512:(h+1)*512] as [128, 512] f32.
            acc = [accp.tile([P, HALF], f32, name=f"acc{i}", tag="acc")
                   for i in range(2 * DB)]
            for i in range(2 * DB):
                nc.vector.memset(acc[i][:], 0.0)

            # ---- stage 2: kv partial, grouped PSUM accumulation ----
            # keys loads on sync, vals loads on gpsimd; first chunks
            # row-split across two DMAs to shorten the start ramp.
            xf_tiles = []
            for g in range(NG):
                kf = []   # per chunk: (tile_cols_0_512, tile_cols_512_1024)
                vf = []
                for c in range(G):
                    if g == 0 and c < 2:
                        # Separate half-tiles for the first chunks:
                        # independent deps, so the first matmul only
                        # waits on a 256KB transfer.
                        kta = kfp.tile([P, HALF], f32r, name="kta",
                                       tag="kth", bufs=4)
                        ktb = kfp.tile([P, HALF], f32r, name="ktb",
                                       tag="kth", bufs=4)
                        vta = vfp.tile([P, HALF], f32r, name="vta",
                                       tag="vth", bufs=4)
                        vtb = vfp.tile([P, HALF], f32r, name="vtb",
                                       tag="vth", bufs=4)
                        nc.sync.dma_start(out=kta[:], in_=ks_r[c][:, :HALF])
                        nc.sync.dma_start(out=vta[:], in_=vs_r[c][:, :HALF])
                        nc.sync.dma_start(out=ktb[:], in_=ks_r[c][:, HALF:])
                        nc.sync.dma_start(out=vtb[:], in_=vs_r[c][:, HALF:])
                        kf.append((kta, ktb))
                        vf.append((vta, vtb))
                    else:
                        kt = kfp.tile([P, D], f32r, name="kt", tag="kt")
                        vt = vfp.tile([P, D], f32r, name="vt", tag="vt")
                        nc.sync.dma_start(out=kt[:], in_=ks_r[g * G + c])
                        nc.sync.dma_start(out=vt[:], in_=vs_r[g * G + c])
                        kf.append((kt, kt))
                        vf.append((vt, vt))
                for h in range(2):
                    e0 = h * HALF
                    pst = [ps.tile([P, HALF], f32, name=f"kv{h}_{j}",
                                   tag="ps") for j in range(DB)]
                    # c-outer keeps the ramp short (first matmuls need
                    # only chunk 0); j-outer in the last group spreads
                    # the final accumulations so the AllReduce triggers
                    # as early as possible.
                    order = (
                        [(j, c) for j in range(DB) for c in range(G)]
                        if g == NG - 1 else
                        [(j, c) for c in range(G) for j in range(DB)])
                    for j, c in order:
                        split = kf[c][0] is not kf[c][1]
                        kt_h = kf[c][j // 4]
                        koff = (j % 4) * P if split else j * P
                        if split:
                            vt_h, voff = vf[c][h], 0
                        else:
                            vt_h, voff = vf[c][0], e0
                        nc.tensor.matmul(
                            pst[j][:],
                            kt_h[:, koff:koff + P],
                            vt_h[:, voff:voff + HALF],
                            start=(c == 0), stop=(c == G - 1))
                    for j in range(DB):
                        nc.vector.tensor_tensor(
                            out=acc[h * DB + j][:],
                            in0=pst[j][:],
                            in1=acc[h * DB + j][:],
                            op=mybir.AluOpType.add)
            # x loads at the tail of the load stream.
            for i in range(TCH):
                xf = xstage.tile([P, D], f32, name="xf", tag="xf")
                nc.sync.dma_start(out=xf[:], in_=xs_r[i])
                xf_tiles.append(xf)

            # ---- AllReduce kv, split by column half ----
            # Cast + bounce each d-block slice as soon as its final
            # accumulation lands, so the collective triggers early.
            kvr = []
            for h in range(2):
                kvev = kvio.tile([P, DB * HALF], bf16, name=f"kvev{h}",
                                 tag="kvio")
                bounce_in = dram.tile([P, DB * HALF], bf16,
                                      name=f"bin{h}", tag="bin")
                bounce_out = dram.tile([P, DB * HALF], bf16,
                                       name=f"bout{h}", tag="bout",
                                       addr_space="Shared")
                for j in range(DB):
                    sl = slice(j * HALF, (j + 1) * HALF)
                    nc.scalar.activation(
                        kvev[:, sl], acc[h * DB + j][:], ACT_COPY)
                    nc.gpsimd.dma_start(out=bounce_in[:, sl],
                                        in_=kvev[:, sl])
                nc.gpsimd.collective_compute(
                    "AllReduce",
                    mybir.AluOpType.add,
                    replica_groups=[list(range(NCORES))],
                    ins=[bounce_in.opt()],
                    outs=[bounce_out.opt()],
                )
                kvr.append(bounce_out)

            # ---- x: PE-transpose, cast to bf16 (fills AR wait) ----
            xT = [xtp.tile([P, T], bf16, name=f"xT{j}", tag="xT")
                  for j in range(DB)]
            for i in range(TCH):
                xf = xf_tiles[i]
                for j in range(DB):
                    pst = ps.tile([P, P], f32, name="pst", tag="ps")
                    nc.tensor.transpose(
                        pst[:], xf[:, j * P:(j + 1) * P], ident[:])
                    nc.vector.tensor_copy(
                        out=xT[j][:, i * P:(i + 1) * P], in_=pst[:])

            # ---- stage 4: out = x @ kv, per column half ----
            for h in range(2):
                kvh = kvio.tile([P, DB * HALF], bf16, name=f"kvr{h}",
                                tag="kvio")
                for j in range(DB):
                    sl = slice(j * HALF, (j + 1) * HALF)
                    nc.gpsimd.dma_start(out=kvh[:, sl], in_=kvr[h][:, sl])
                for i in range(TCH):
                    po = ps.tile([P, HALF], f32, name="po", tag="ps")
                    for j in range(DB):
                        nc.tensor.matmul(
                            po[:],
                            xT[j][:, i * P:(i + 1) * P],
                            kvh[:, j * HALF:(j + 1) * HALF],
                            start=(j == 0), stop=(j == DB - 1))
                    ob = outp.tile([P, HALF], f32, name="ob", tag="ob")
                    nc.scalar.activation(ob[:], po[:], ACT_COPY)
                    nc.scalar.dma_start(
                        out=out_d.ap()[i * P:(i + 1) * P,
                                       h * HALF:(h + 1) * HALF],
                        in_=ob[:])

    nc.compile()
    return nc


def _get_nc():
    if "nc" not in _CACHE:
        _CACHE["nc"] = _build_nc()
    return _CACHE["nc"]


def kernel(**inputs):
    from concourse.bass_utils import run_bass_kernel_spmd

    x = np.ascontiguousarray(np.asarray(inputs["x"], dtype=np.float32))
    keys = np.ascontiguousarray(np.asarray(inputs["keys"], dtype=np.float32))
    vals = np.ascontiguousarray(np.asarray(inputs["vals"], dtype=np.float32))
    xf = x.reshape(B * S, D)

    nc = _get_nc()
    in_maps = []
    for c in range(NCORES):
        in_maps.append({
            "xs": xf[c * T:(c + 1) * T],
            "ks": keys[c * KM:(c + 1) * KM],
            "vs": vals[c * KM:(c + 1) * KM],
        })
    res = run_bass_kernel_spmd(nc, in_maps, list(range(NCORES)))
    out = np.concatenate([res.results[c]["out"] for c in range(NCORES)],
                         axis=0)
    return out.reshape(B, S, D).astype(np.float32)
